# revision 9
# baseline (speedup 1.0000x reference)
# Trainium2 Bass kernel for nn_Affinity: M[i,j] = w2 . relu(hx[i] + hy[j] + b1) + b2
# where hx = (X @ W_sr.T) @ W1x.T, hy = (Y @ W_tg.T) @ W1y.T.
#
# Sharding: rows of X (N1=512) split across 8 cores, 64 rows each; Y and all
# weights replicated. Each core computes a [64, 512] tile of M.
#
# Host passes X.T shard, Y.T, W1x.T, W1y.T (layout prep only); W_sr/W_tg are
# used in natural layout as matmul stationaries (lhsT), so on-device the chain
#   AxT = W_sr.T @ W1xT   (i.e. AxT[c,h] = sum_c' Wsr[c',c] W1x[h,c'])
#   hxT = AxT.T @ XT      (+ b1 folded in during PSUM->SBUF copy-out)
#   AyT = W_tg.T @ W1yT ; hyT = AyT.T @ YT   (cast to bf16)
# needs no transposes at all.
#
# Per-core layout: h (hidden, 512) lives on SBUF partitions in 4 blocks of 128.
#   hyT[hb] : [128h, 512j]  (bf16)   hxT[hb] : [128h, 64i]  (f32, b1 folded in)
# Main loop over i-groups of 4: relu tiles r = relu(hyT[hb] + hxT[hb][:,i])
# produced on DVE (tensor_scalar add+max) and ACT (activation Relu+bias),
# contracted with w2 on the PE (M=32 replicated-w2 matmuls at col positions
# 0/32/64/96 -> 4 concurrent strips), accumulated over hb in PSUM, then
# b2-add + PSUM->SBUF copy and a strided-partition DMA to DRAM.

import sys

try:
    import concourse  # noqa: F401
except ImportError:
    sys.path.insert(0, "/opt/trn_rl_repo")

import numpy as np

import concourse.mybir as mybir
from concourse import bacc
from concourse.bass import ds, ts
from concourse.tile import TileContext

F32 = mybir.dt.float32
BF16 = mybir.dt.bfloat16

N1, N2, C, H = 512, 512, 256, 512
NCORES = 8
ISH = N1 // NCORES          # 64 rows of X per core
HB = H // 128               # 4 h blocks
CB = C // 128               # 2 c blocks
NGROUP = ISH // 4           # 16 i-groups of 4


def build_program():
    nc = bacc.Bacc("TRN2", target_bir_lowering=False, debug=False)

    XT = nc.dram_tensor("XT", [C, ISH], F32, kind="ExternalInput")
    YT = nc.dram_tensor("YT", [C, N2], F32, kind="ExternalInput")
    Wsr = nc.dram_tensor("Wsr", [C, C], F32, kind="ExternalInput")
    Wtg = nc.dram_tensor("Wtg", [C, C], F32, kind="ExternalInput")
    W1xT = nc.dram_tensor("W1xT", [C, H], F32, kind="ExternalInput")
    W1yT = nc.dram_tensor("W1yT", [C, H], F32, kind="ExternalInput")
    w2rep = nc.dram_tensor("w2rep", [128, HB * 32], F32, kind="ExternalInput")
    b1c = nc.dram_tensor("b1c", [128, HB], F32, kind="ExternalInput")
    b2c = nc.dram_tensor("b2c", [128, 1], F32, kind="ExternalInput")
    Msh = nc.dram_tensor("Msh", [ISH, N2], F32, kind="ExternalOutput")

    AF = mybir.ActivationFunctionType
    OP = mybir.AluOpType

    with TileContext(nc) as tc:
        with tc.tile_pool(name="const", bufs=1) as const, \
             tc.tile_pool(name="rt", bufs=8) as rp, \
             tc.tile_pool(name="ep", bufs=3) as epp, \
             tc.tile_pool(name="pst", bufs=3, space="PSUM") as pst, \
             tc.tile_pool(name="psm", bufs=3, space="PSUM") as psm:

            # ---------- input DMAs (Y-side first: longest chain) ----------
            def load(name, dram, rows, cols, dtype=F32, dma=None):
                tiles = []
                for b in range(rows // 128):
                    t = const.tile([128, cols], dtype, tag=f"{name}{b}",
                                   name=f"{name}{b}")
                    (dma or nc.sync).dma_start(t[:, :], dram[ts(b, 128), :])
                    tiles.append(t)
                return tiles

            yt = load("yt", YT, C, N2)
            w1yt = load("w1yt", W1yT, C, H)
            wtg = load("wtg", Wtg, C, C)
            w2sb = const.tile([128, HB * 32], BF16, tag="w2sb")
            nc.gpsimd.dma_start(w2sb[:, :], w2rep[:, :])
            xt = load("xt", XT, C, ISH)
            w1xt = load("w1xt", W1xT, C, H)
            wsr = load("wsr", Wsr, C, C)
            b1sb = const.tile([128, HB], F32, tag="b1")
            nc.sync.dma_start(b1sb[:, :], b1c[:, :])
            b2b = const.tile([128, 1], F32, tag="b2")
            nc.sync.dma_start(b2b[:, :], b2c[:, :])

            # ---------- chain matmuls ----------
            copy_alt = [nc.vector.tensor_copy, nc.scalar.copy]
            cnt = 0

            def alt():
                nonlocal cnt
                cnt += 1
                return copy_alt[cnt % 2]

            # AyT[c, h] = sum_c' Wtg[c', c] * W1y[h, c']  (lhsT = Wtg natural)
            AyT = [const.tile([128, H], F32, tag=f"ay{mb}", name=f"ay{mb}")
                   for mb in range(CB)]
            for mb in range(CB):
                ps = pst.tile([128, 512], F32, tag="pst")
                for kb in range(CB):
                    nc.tensor.matmul(ps[:, :], wtg[kb][:, ts(mb, 128)],
                                     w1yt[kb][:, :],
                                     start=(kb == 0), stop=(kb == CB - 1))
                alt()(AyT[mb][:, :], ps[:, :])
            # hyT[h, j] = sum_c AyT[c, h(mb)] * YT[c, j]  (cast to bf16)
            hyT = [const.tile([128, N2], BF16, tag=f"hy{mb}", name=f"hy{mb}")
                   for mb in range(HB)]
            for mb in range(HB):
                ps = pst.tile([128, 512], F32, tag="pst")
                for kb in range(CB):
                    nc.tensor.matmul(ps[:, :], AyT[kb][:, ts(mb, 128)],
                                     yt[kb][:, :],
                                     start=(kb == 0), stop=(kb == CB - 1))
                alt()(hyT[mb][:, :], ps[:, :])
            # X side
            AxT = [const.tile([128, H], F32, tag=f"ax{mb}", name=f"ax{mb}")
                   for mb in range(CB)]
            for mb in range(CB):
                ps = pst.tile([128, 512], F32, tag="pst")
                for kb in range(CB):
                    nc.tensor.matmul(ps[:, :], wsr[kb][:, ts(mb, 128)],
                                     w1xt[kb][:, :],
                                     start=(kb == 0), stop=(kb == CB - 1))
                alt()(AxT[mb][:, :], ps[:, :])
            # hxT[h, i] = sum_c AxT[c, h(mb)] * XT[c, i]; fold b1 on copy-out
            hxT = [const.tile([128, ISH], F32, tag=f"hx{mb}", name=f"hx{mb}")
                   for mb in range(HB)]
            for mb in range(HB):
                ps = pst.tile([128, 512], F32, tag="pst")
                for kb in range(CB):
                    nc.tensor.matmul(ps[:, 0:ISH], AxT[kb][:, ts(mb, 128)],
                                     xt[kb][:, :],
                                     start=(kb == 0), stop=(kb == CB - 1))
                nc.vector.tensor_scalar_add(hxT[mb][:, :], ps[:, 0:ISH],
                                            b1sb[:, ds(mb, 1)])

            # ---------- main loop ----------
            for g in range(NGROUP):
                psM = psm.tile([128, N2], F32, tag="psM")
                for hb in range(HB):
                    for q in range(4):
                        i = 4 * g + q
                        rt = rp.tile([128, N2], BF16, tag="rt")
                        idx = g * 16 + hb * 4 + q
                        if idx % 4 < 3:
                            nc.vector.tensor_scalar(
                                rt[:, :], hyT[hb][:, :], hxT[hb][:, ds(i, 1)],
                                0.0, op0=OP.add, op1=OP.max)
                        else:
                            nc.scalar.activation(
                                rt[:, :], hyT[hb][:, :], AF.Relu,
                                bias=hxT[hb][:, ds(i, 1)], scale=1.0)
                        nc.tensor.matmul(
                            psM[ds(32 * q, 32), :], w2sb[:, ts(hb, 32)],
                            rt[:, :],
                            start=(hb == 0), stop=(hb == HB - 1),
                            tile_position=(0, 32 * q), skip_group_check=True)
                ep = epp.tile([128, N2], F32, tag="ep")
                nc.scalar.activation(ep[:, :], psM[:, :], AF.Identity,
                                     bias=b2b[:, 0:1], scale=1.0)
                nc.sync.dma_start(Msh[ds(4 * g, 4), :], ep[0:97:32, :])

    nc.compile()
    return nc


_CACHE = {}


def _get_program():
    if "nc" not in _CACHE:
        _CACHE["nc"] = build_program()
    return _CACHE["nc"]


def make_in_maps(inputs):
    f32c = lambda a: np.ascontiguousarray(np.asarray(a, dtype=np.float32))
    X = f32c(inputs["X"])
    w2 = f32c(inputs["w2"]).reshape(H)
    # w2rep[p, hb*32 + r] = w2[hb*128 + p]
    w2rep = np.ascontiguousarray(
        np.broadcast_to(w2.reshape(HB, 128).T[:, :, None],
                        (128, HB, 32)).reshape(128, HB * 32))
    b1 = f32c(inputs["b1"]).reshape(H)
    in_common = {
        "YT": f32c(inputs["Y"].T),
        "Wsr": f32c(inputs["W_sr"]),
        "Wtg": f32c(inputs["W_tg"]),
        "W1xT": f32c(np.asarray(inputs["W1"])[:, :C].T),
        "W1yT": f32c(np.asarray(inputs["W1"])[:, C:].T),
        "w2rep": w2rep,
        "b1c": f32c(b1.reshape(HB, 128).T),
        "b2c": np.full((128, 1), np.float32(np.asarray(inputs["b2"]).reshape(-1)[0]),
                       dtype=np.float32),
    }
    return [
        {"XT": f32c(X[c * ISH:(c + 1) * ISH].T), **in_common}
        for c in range(NCORES)
    ]


def run(inputs, trace=False):
    from concourse.bass_utils import run_bass_kernel_spmd

    nc = _get_program()
    in_maps = make_in_maps(inputs)
    res = run_bass_kernel_spmd(nc, in_maps, core_ids=list(range(NCORES)),
                               trace=trace)
    out = np.concatenate([res.results[c]["Msh"] for c in range(NCORES)], axis=0)
    return out.astype(np.float32), res


def kernel(**inputs):
    out, _ = run(inputs, trace=False)
    return out
